# revision 1
# baseline (speedup 1.0000x reference)
"""3-layer GCN (PyG GCNConv-style) on 8 Trainium2 NeuronCores.

Strategy (graph/data parallel, per the sharding hint):
  - Nodes partitioned contiguously: 6272 per core (49 windows x 128 nodes;
    node n is owned by core n // 6272). Edges assigned to the core owning
    their destination; self-loops added host-side as ordinary edges.
  - Per layer: each core computes xw = h_own @ W for its own nodes (49
    matmuls off a feature-major hT kept in SBUF), AllGathers the full
    [50176, 64] xw table, then gathers xw[src] rows for its edges with
    dma_gather and performs the normalized scatter-add as matmul-
    accumulates into PSUM: for each 128-edge tile, a [128e x 128dst]
    "value matrix" (norm[e] at column dstcol[e], built on the Vector
    engine with a fused iota/is_equal/mult op) multiplies the gathered
    messages, accumulating a 128-node window's segment sum.
  - dma_gather uses int16 indices, so the 50176-row xw table is split in
    half (rows < 25088 = cores 0-3); each window's edge tiles are grouped
    lo-half first, then hi-half, and gathers run per half over groups of
    consecutive windows.
  - Window epilogue: +bias, sigmoid, agent-row tap (stride-4 partitions),
    and a PE transpose back into the next layer's feature-major hT.

Host-side work is limited to graph index preprocessing (degree counts,
edge normalization constants, sorting/padding edges by destination
window) and final output assembly.
"""

import sys

sys.path.insert(0, "/opt/trn_rl_repo")

import numpy as np

N_NODES = 50000
D = 64
N_CORES = 8
WSZ = 128               # dst-window size (PSUM partition dim)
NW = 49                 # windows per core
NPC = NW * WSZ          # 6272 padded nodes per core (50176 total >= 50000)
LO_ROWS = 4 * NPC       # 25088: table-half split (int16 index limit)
GRP = 2                 # windows per gather group


def _preprocess(edge_index):
    """Degree/norm computation and edge layout.

    Returns per-core device arrays plus the uniform tile schedule (shared
    by all cores: the program is SPMD, so tile counts per (window, half)
    are maxed over cores and padded with norm=0 edges).
    """
    src = np.asarray(edge_index[0], dtype=np.int64)
    dst = np.asarray(edge_index[1], dtype=np.int64)

    deg = np.bincount(dst, minlength=N_NODES).astype(np.float32) + 1.0
    dinv = (1.0 / np.sqrt(deg)).astype(np.float32)

    loop = np.arange(N_NODES, dtype=np.int64)
    s_all = np.concatenate([src, loop])
    d_all = np.concatenate([dst, loop])
    nrm = np.concatenate([dinv[src] * dinv[dst], dinv * dinv]).astype(np.float32)

    core = d_all // NPC
    local = d_all - core * NPC
    win = local // WSZ
    col = (local % WSZ).astype(np.float32)
    half = (s_all >= LO_ROWS).astype(np.int64)

    # group edges by (core, window, half)
    key = (core * NW + win) * 2 + half
    order = np.argsort(key, kind="stable")
    key_sorted = key[order]
    bounds = np.searchsorted(key_sorted, np.arange(N_CORES * NW * 2 + 1))
    cnt = (bounds[1:] - bounds[:-1]).reshape(N_CORES, NW, 2)

    # uniform tiles per (window, half), maxed over cores
    n_th = -(-cnt.max(axis=0) // 128)           # [NW, 2]
    n_th[:, 0] = np.maximum(n_th[:, 0], 1)      # >=1 tile so PSUM gets start=True
    T = int(n_th.sum())

    # tile stream: groups of GRP windows; within a group all lo tiles
    # (w ascending), then all hi tiles
    tile_win = []       # window of tile t
    tile_half = []
    runs = []           # (start_tile, n_tiles, half)
    win_tile_base = np.zeros((NW, 2), np.int64)
    for g0 in range(0, NW, GRP):
        ws = range(g0, min(g0 + GRP, NW))
        for h in (0, 1):
            r0 = len(tile_win)
            for w in ws:
                win_tile_base[w, h] = len(tile_win)
                tile_win += [w] * int(n_th[w, h])
                tile_half += [h] * int(n_th[w, h])
            if len(tile_win) > r0:
                runs.append((r0, len(tile_win) - r0, h))
    tile_win = np.asarray(tile_win)
    tile_half = np.asarray(tile_half)
    assert len(tile_win) == T

    # first/last tile of each window (for PSUM start/stop)
    win_first = np.full(NW, -1, np.int64)
    win_last = np.full(NW, -1, np.int64)
    for t in range(T):
        w = tile_win[t]
        if win_first[w] < 0:
            win_first[w] = t
        win_last[w] = t

    # fill per-core edge arrays
    idx_flat = np.zeros((N_CORES, T * 128), np.int16)
    col_arr = np.zeros((N_CORES, 128, T), np.float32)
    nrm_arr = np.zeros((N_CORES, 128, T), np.float32)
    for c in range(N_CORES):
        for w in range(NW):
            for h in (0, 1):
                gidx = (c * NW + w) * 2 + h
                e0, e1 = bounds[gidx], bounds[gidx + 1]
                n = e1 - e0
                if n == 0:
                    continue
                sel = order[e0:e1]
                base = win_tile_base[w, h] * 128
                pos = base + np.arange(n)
                idx_flat[c, pos] = (s_all[sel] - h * LO_ROWS).astype(np.int16)
                tt = pos // 128
                pp = pos % 128
                col_arr[c, pp, tt] = col[sel]
                nrm_arr[c, pp, tt] = nrm[sel]

    # wrap indices for dma_gather: [128, T*8] int16,
    # arr[p, t*8+c] = idx[t*128 + c*16 + (p % 16)]
    w16 = idx_flat.reshape(N_CORES, T, 8, 16).transpose(0, 3, 1, 2).reshape(
        N_CORES, 16, T * 8)
    idx_arr = np.tile(w16, (1, 8, 1))           # [N_CORES, 128, T*8]

    sched = dict(T=T, runs=runs, tile_win=tile_win, tile_half=tile_half,
                 win_first=win_first, win_last=win_last)
    return idx_arr, col_arr, nrm_arr, sched


def _build_program(sched, repeat=1):
    import os
    VAR = set(os.environ.get("KVAR", "").split(","))
    import concourse.bass as bass
    import concourse.bacc as bacc
    import concourse.tile as tile
    from concourse import mybir

    f32 = mybir.dt.float32
    i16 = mybir.dt.int16

    T = sched["T"]
    runs = sched["runs"]
    if "sp1" in VAR:
        runs = [(t0 + o, min(8, nt - o), h)
                for (t0, nt, h) in runs for o in range(0, nt, 8)]
    tile_win = sched["tile_win"]
    win_first = sched["win_first"]
    win_last = sched["win_last"]
    max_run = max(n for _, n, _ in runs)

    nsq = 4 if "q4" in VAR else 1
    nc = bacc.Bacc("TRN2", target_bir_lowering=False, debug=False,
                   num_devices=N_CORES, num_swdge_queues=nsq)

    xT_own = nc.dram_tensor("xT_own", [64, NPC], f32, kind="ExternalInput")
    src_idx = nc.dram_tensor("src_idx", [128, T * 8], i16, kind="ExternalInput")
    dstcol = nc.dram_tensor("dstcol", [128, T], f32, kind="ExternalInput")
    normv = nc.dram_tensor("normv", [128, T], f32, kind="ExternalInput")
    Wmat = nc.dram_tensor("Wmat", [3, 64, 64], f32, kind="ExternalInput")
    bias_bc = nc.dram_tensor("bias_bc", [3, 128, 64], f32, kind="ExternalInput")
    iota_in = nc.dram_tensor("iota", [128, 128], f32, kind="ExternalInput")
    ident_in = nc.dram_tensor("ident", [128, 128], f32, kind="ExternalInput")
    agents = nc.dram_tensor("agents_out", [3, NW * 32, 64], f32,
                            kind="ExternalOutput")

    with tile.TileContext(nc) as tc:
        with (
            tc.tile_pool(name="const", bufs=1) as constp,
            tc.tile_pool(name="hT", bufs=2) as hTp,
            tc.tile_pool(name="msg", bufs=3) as msgp,
            tc.tile_pool(name="vm", bufs=4) as vmp,
            tc.tile_pool(name="small", bufs=4) as smallp,
            tc.tile_pool(name="ps_seg", bufs=3, space="PSUM") as ps_seg,
            tc.tile_pool(name="ps_xw", bufs=2, space="PSUM") as ps_xw,
            tc.tile_pool(name="ps_tr", bufs=2, space="PSUM") as ps_tr,
            tc.tile_pool(name="dram_ag", bufs=2, space="DRAM") as dram_ag,
            tc.tile_pool(name="dram_xw", bufs=1, space="DRAM") as dram_xw,
        ):
            meta_idx = constp.tile([128, T * 8], i16)
            meta_col = constp.tile([128, T], f32)
            meta_nrm = constp.tile([128, T], f32)
            nc.sync.dma_start(out=meta_idx[:], in_=src_idx[:, :])
            nc.sync.dma_start(out=meta_col[:], in_=dstcol[:, :])
            nc.sync.dma_start(out=meta_nrm[:], in_=normv[:, :])
            iota_t = constp.tile([128, 128], f32)
            ident_t = constp.tile([128, 128], f32)
            nc.sync.dma_start(out=iota_t[:], in_=iota_in[:, :])
            nc.sync.dma_start(out=ident_t[:], in_=ident_in[:, :])
            w_tiles = []
            b_tiles = []
            for l in range(3):
                wt = constp.tile([64, 64], f32, name=f"w{l}")
                bt = constp.tile([128, 64], f32, name=f"b{l}")
                nc.sync.dma_start(out=wt[:], in_=Wmat[l, :, :])
                nc.sync.dma_start(out=bt[:], in_=bias_bc[l, :, :])
                w_tiles.append(wt)
                b_tiles.append(bt)

            def emit(rep):
                hT_cur = hTp.tile([64, NPC], f32, tag="hT", name=f"r{rep}hT0")
                nc.sync.dma_start(out=hT_cur[:], in_=xT_own[:, :])
                for l in range(3):
                    # ---- own-shard linear: xw_own = h_own @ W_l ----
                    ag_t = dram_ag.tile([NPC, 64], f32, tag="ag", name=f"r{rep}ag{l}")
                    for i in range(NW):
                        ps = ps_xw.tile([128, 64], f32, tag="psxw", name=f"r{rep}psxw{l}_{i}")
                        nc.tensor.matmul(
                            out=ps[:],
                            lhsT=hT_cur[:, i * 128:(i + 1) * 128],
                            rhs=w_tiles[l][:],
                            start=True, stop=True,
                        )
                        st = smallp.tile([128, 64], f32, tag="xwst", name=f"r{rep}st{l}_{i}")
                        nc.vector.tensor_copy(out=st[:], in_=ps[:])
                        nc.sync.dma_start(out=ag_t[i * 128:(i + 1) * 128, :], in_=st[:])

                    xw_full = dram_xw.tile(
                        [N_CORES, NPC, 64], f32, tag="xwf",
                        addr_space=("Local" if "noshared" in VAR else "Shared"),
                        name=f"r{rep}xwf{l}")
                    if "nocoll" in VAR:
                        nc.sync.dma_start(out=xw_full[0, :, :], in_=ag_t[:])
                    else:
                        nc.gpsimd.collective_compute(
                            "AllGather",
                            mybir.AluOpType.bypass,
                            replica_groups=[list(range(N_CORES))],
                            ins=[ag_t.opt()],
                            outs=[xw_full.opt()],
                        )
                    xw_flat = xw_full[:].rearrange("a b c -> (a b) c")

                    if l < 2:
                        hT_next = hTp.tile([64, NPC], f32, tag="hT", name=f"r{rep}hT{l + 1}")
                    else:
                        hT_next = None

                    # ---- gather + windowed segment-sum ----
                    if "noseg" in VAR:
                        if hT_next is not None:
                            nc.sync.dma_start(out=hT_next[:], in_=xT_own[:, :])
                        hT_cur = hT_next
                        continue
                    win_ps = {}
                    for r, (t0, nt, h) in enumerate(runs):
                        msg = msgp.tile([128, max_run * 64], f32, tag="msg",
                                        name=f"r{rep}msg{l}_{r}")
                        src_half = (xw_flat[0:LO_ROWS, :] if h == 0
                                    else xw_flat[LO_ROWS:2 * LO_ROWS, :])
                        if "nogather" not in VAR:
                            nc.gpsimd.dma_gather(
                                out_ap=msg[:, :nt * 64].rearrange("p (k f) -> p k f", f=64),
                                in_ap=src_half,
                                idxs_ap=meta_idx[:, t0 * 8:(t0 + nt) * 8],
                                num_idxs=nt * 128,
                                num_idxs_reg=nt * 128,
                                elem_size=64,
                                single_packet=("sp1" in VAR),
                                queue_num=(r % nsq),
                            )
                        for j in range(nt):
                            t = t0 + j
                            w = int(tile_win[t])
                            first = (t == win_first[w])
                            last = (t == win_last[w])
                            if first:
                                win_ps[w] = ps_seg.tile([128, 64], f32, tag="seg",
                                                        name=f"r{rep}seg{l}_{w}")
                            cur_ps = win_ps[w]
                            vm = vmp.tile([128, 128], f32, tag="vm", name=f"r{rep}vm{l}_{t}")
                            if "novm" not in VAR:
                                nc.vector.tensor_scalar(
                                    out=vm[:],
                                    in0=iota_t[:],
                                    scalar1=meta_col[:, t:t + 1],
                                    scalar2=meta_nrm[:, t:t + 1],
                                    op0=mybir.AluOpType.is_equal,
                                    op1=mybir.AluOpType.mult,
                                )
                            if "nomm" in VAR:
                                if first:
                                    nc.vector.tensor_copy(out=cur_ps[:], in_=b_tiles[l][:])
                                if last and "nogather" not in VAR:
                                    nc.vector.tensor_copy(out=vm[:, 0:64], in_=msg[:, j * 64:(j + 1) * 64])
                            if "nomm" not in VAR:
                                nc.tensor.matmul(
                                    out=cur_ps[:],
                                    lhsT=vm[:],
                                    rhs=msg[:, j * 64:(j + 1) * 64],
                                    start=first, stop=last,
                                )
                            if last:
                                hwin = smallp.tile([128, 64], f32, tag="hwin",
                                                   name=f"r{rep}hw{l}_{w}")
                                nc.vector.tensor_add(out=hwin[:], in0=cur_ps[:],
                                                     in1=b_tiles[l][:])
                                nc.scalar.activation(
                                    out=hwin[:], in_=hwin[:],
                                    func=mybir.ActivationFunctionType.Sigmoid,
                                )
                                nc.sync.dma_start(
                                    out=agents[l, w * 32:(w + 1) * 32, :],
                                    in_=hwin[0:128:4, :],
                                )
                                if hT_next is not None:
                                    pt = ps_tr.tile([64, 128], f32, tag="tr",
                                                    name=f"r{rep}tr{l}_{w}")
                                    nc.tensor.transpose(out=pt[:], in_=hwin[:],
                                                        identity=ident_t[:])
                                    nc.vector.tensor_copy(
                                        out=hT_next[:, w * 128:(w + 1) * 128],
                                        in_=pt[:],
                                    )
                    hT_cur = hT_next


            for rep in range(repeat):
                emit(rep)

    nc.compile()
    return nc


def kernel(**inputs):
    from concourse import bass_utils

    x = np.asarray(inputs["x"], dtype=np.float32)
    edge_index = np.asarray(inputs["edge_index"])
    agent_idx = np.asarray(inputs["agent_idx"], dtype=np.int64)
    Ws = [np.asarray(inputs[f"W{i}"], dtype=np.float32) for i in range(3)]
    bs = [np.asarray(inputs[f"b{i}"], dtype=np.float32) for i in range(3)]

    idx_arr, col_arr, nrm_arr, sched = _preprocess(edge_index)

    nc = _build_program(sched)

    xpad = np.zeros((N_CORES * NPC, D), np.float32)
    xpad[:N_NODES] = x
    Wstack = np.ascontiguousarray(np.stack(Ws))
    bias_stack = np.ascontiguousarray(
        np.stack([np.tile(b[None, :], (128, 1)) for b in bs]))
    iota = np.tile(np.arange(128, dtype=np.float32)[None, :], (128, 1))
    ident = np.eye(128, dtype=np.float32)

    in_maps = []
    for c in range(N_CORES):
        in_maps.append({
            "xT_own": np.ascontiguousarray(xpad[c * NPC:(c + 1) * NPC].T),
            "src_idx": np.ascontiguousarray(idx_arr[c]),
            "dstcol": np.ascontiguousarray(col_arr[c]),
            "normv": np.ascontiguousarray(nrm_arr[c]),
            "Wmat": Wstack,
            "bias_bc": bias_stack,
            "iota": iota,
            "ident": ident,
        })

    res = bass_utils.run_bass_kernel_spmd(
        nc, in_maps, core_ids=list(range(N_CORES)))

    taps = np.stack([res.results[c]["agents_out"] for c in range(N_CORES)])
    # taps[c, l, r, :] = h_l for node (c*NPC + 4*r)
    n_agents = agent_idx.shape[0]
    out = np.empty((n_agents, 3 * D), np.float32)
    c_of = agent_idx // NPC
    r_of = (agent_idx % NPC) // 4
    for l in range(3):
        out[:, l * D:(l + 1) * D] = taps[c_of, l, r_of, :]
    return out



# revision 9
# speedup vs baseline: 1.3233x; 1.3233x over previous
"""3-layer GCN (PyG GCNConv-style) on 8 Trainium2 NeuronCores — v2.

Strategy (graph/data parallel; nodes sharded by destination core):
  - Nodes partitioned contiguously: 6272 per core (49 windows x 128). Edges
    (incl. host-added self-loops) are owned by the core owning their dst.
  - Normalization is folded away: the gathered xw table holds
    dinv[src] * (h @ W) rows, and the window epilogue applies the dinv[dst]
    factor as the per-partition `scale` of the sigmoid activation. The bias
    is injected into PSUM via a K=1 rank-1 matmul with a sqrt(deg[dst])
    column so it survives the later dinv[dst] scaling.
  - The per-edge one-hot scatter matrices are graph-static: built ONCE on
    the host in fp8 (exact for 0/1) and streamed from DRAM each layer,
    freeing the Vector engine entirely (v1 spent 85% of the span there).
  - Gathers use int16 indices, so the 50176-row xw table is split into
    half-tables A (local row < 3200, 25600 rows) and B (24576 rows). Each
    layer runs two phases: phase A processes every window's A-half tiles
    (window-major, one live PSUM bank, parked to SBUF in bf16 at window
    close), phase B re-injects the parked sum via an identity matmul and
    finishes the window (sigmoid epilogue, agent tap, PE transpose into the
    next layer's hT). Phases are split into ~128-tile gather chunks — the
    per-call Q7 descriptor-gen cost is ~20us flat, so calls are few and big.
  - Per layer: 49 own-shard bf16 matmuls -> ScalarE evac (x dinv, cast bf16)
    -> one DMA into the padded [6272, 128]-bf16 shard -> two AllGathers
    (A-half first so phase-A gathers start sooner).

Host-side work: graph preprocessing (degrees, edge layout, one-hot tiles)
and final output assembly.
"""

import sys

sys.path.insert(0, "/opt/trn_rl_repo")

import numpy as np
import ml_dtypes

N_NODES = 50000
D = 64
N_CORES = 8
WSZ = 128               # dst-window size (PSUM partition dim)
NW = 49                 # windows per core
NPC = NW * WSZ          # 6272 padded nodes per core (50176 total >= 50000)
HALF_A = 3200           # local rows < HALF_A -> table A (25 windows' rows)
HALF_B = NPC - HALF_A   # 3072 rows -> table B
ROWS_A = N_CORES * HALF_A   # 25600 (< 32767, int16-addressable)
ROWS_B = N_CORES * HALF_B   # 24576
CMAX = 128              # max tiles per gather chunk (SBUF-bound)


def _preprocess(edge_index):
    """Edge layout + one-hot scatter tiles.

    Tile stream: [phase A: w0..w48, each window's A-half tiles]
                 [phase B: w0..w48, each window's B-half tiles].
    SPMD: tile counts per (window, half) are maxed over cores; padded slots
    get all-zero one-hot rows so they contribute nothing.
    """
    src = np.asarray(edge_index[0], dtype=np.int64)
    dst = np.asarray(edge_index[1], dtype=np.int64)

    deg = np.bincount(dst, minlength=N_NODES).astype(np.float32) + 1.0
    dinv = (1.0 / np.sqrt(deg)).astype(np.float32)
    sqdeg = np.sqrt(deg).astype(np.float32)

    loop = np.arange(N_NODES, dtype=np.int64)
    s_all = np.concatenate([src, loop])
    d_all = np.concatenate([dst, loop])

    core = d_all // NPC
    local = d_all - core * NPC
    win = local // WSZ
    col = local % WSZ

    s_core = s_all // NPC
    s_loc = s_all - s_core * NPC
    half = (s_loc >= HALF_A).astype(np.int64)
    idx16 = np.where(half == 0, s_core * HALF_A + s_loc,
                     s_core * HALF_B + (s_loc - HALF_A))

    # group edges by (core, half, win)
    key = (core * 2 + half) * NW + win
    nkey = N_CORES * 2 * NW
    order = np.argsort(key, kind="stable")
    key_sorted = key[order]
    bounds = np.searchsorted(key_sorted, np.arange(nkey + 1))
    cnt = (bounds[1:] - bounds[:-1]).reshape(N_CORES, 2, NW)

    # uniform tiles per (half, win), maxed over cores
    n_th = -(-cnt.max(axis=0) // WSZ)               # [2, NW]
    T = int(n_th.sum())

    # tile stream + gather chunks (runs)
    tile_win = []
    runs = []                                       # (t0, nt, half)
    win_tile_base = np.zeros((2, NW), np.int64)
    for h in (0, 1):
        p0 = len(tile_win)
        for w in range(NW):
            win_tile_base[h, w] = len(tile_win)
            tile_win += [w] * int(n_th[h, w])
        np_h = len(tile_win) - p0                   # tiles in this phase
        if np_h == 0:
            continue
        n_chunks = -(-np_h // CMAX)
        splits = np.linspace(p0, p0 + np_h, n_chunks + 1).astype(np.int64)
        for a, b in zip(splits[:-1], splits[1:]):
            if b > a:
                runs.append((int(a), int(b - a), h))
    tile_win = np.asarray(tile_win)
    assert len(tile_win) == T
    max_run = max(nt for _, nt, _ in runs)

    # per-window first/last tile within each phase (-1 if none)
    wfirst = np.full((2, NW), -1, np.int64)
    wlast = np.full((2, NW), -1, np.int64)
    for h in (0, 1):
        for w in range(NW):
            if n_th[h, w] > 0:
                wfirst[h, w] = win_tile_base[h, w]
                wlast[h, w] = win_tile_base[h, w] + n_th[h, w] - 1

    # per-core edge slot arrays
    idx_flat = np.zeros((N_CORES, T * WSZ), np.int16)
    vm8 = np.zeros((N_CORES, WSZ, T * WSZ), ml_dtypes.float8_e4m3)
    for c in range(N_CORES):
        for h in (0, 1):
            for w in range(NW):
                gidx = (c * 2 + h) * NW + w
                e0, e1 = bounds[gidx], bounds[gidx + 1]
                n = e1 - e0
                if n == 0:
                    continue
                sel = order[e0:e1]
                base = win_tile_base[h, w] * WSZ
                pos = base + np.arange(n)
                idx_flat[c, pos] = idx16[sel].astype(np.int16)
                tt = pos // WSZ
                pp = pos % WSZ
                vm8[c, pp, tt * WSZ + col[sel]] = 1.0

    # wrap indices for dma_gather: [128, T*8] int16,
    # arr[p, t*8 + cc] = idx[t*128 + cc*16 + (p % 16)]
    w16 = idx_flat.reshape(N_CORES, T, 8, 16).transpose(0, 3, 1, 2).reshape(
        N_CORES, 16, T * 8)
    idx_arr = np.tile(w16, (1, 8, 1))               # [N_CORES, 128, T*8]

    # per-core epilogue scale layouts
    dinv_pad = np.ones(N_CORES * NPC, np.float32)
    sqdeg_pad = np.ones(N_CORES * NPC, np.float32)
    dinv_pad[:N_NODES] = dinv
    sqdeg_pad[:N_NODES] = sqdeg
    dinv_own = dinv_pad.reshape(N_CORES, NW, WSZ).transpose(0, 2, 1).copy()
    sqdeg_own = sqdeg_pad.reshape(N_CORES, 1, NPC).copy()

    sched = dict(T=T, runs=runs, tile_win=tile_win, n_th=n_th,
                 wfirst=wfirst, wlast=wlast, max_run=max_run)
    return idx_arr, vm8, dinv_own, sqdeg_own, sched


def _build_program(sched):
    import os
    VAR = set(os.environ.get("KVAR", "").split(","))
    import concourse.bass as bass
    import concourse.bacc as bacc
    import concourse.tile as tile
    from concourse import mybir

    f32 = mybir.dt.float32
    bf16 = mybir.dt.bfloat16
    fp8 = mybir.dt.float8e4
    i16 = mybir.dt.int16

    T = sched["T"]
    runs = sched["runs"]
    tile_win = sched["tile_win"]
    n_th = sched["n_th"]
    wfirst = sched["wfirst"]
    wlast = sched["wlast"]
    max_run = sched["max_run"]

    nsq = 4
    nc = bacc.Bacc("TRN2", target_bir_lowering=False, debug=False,
                   num_devices=N_CORES, num_swdge_queues=nsq)

    xT_own = nc.dram_tensor("xT_own", [64, NPC], bf16, kind="ExternalInput")
    src_idx = nc.dram_tensor("src_idx", [128, T * 8], i16, kind="ExternalInput")
    vm_in = nc.dram_tensor("vm8", [128, T * 128], fp8, kind="ExternalInput")
    dinv_in = nc.dram_tensor("dinv_own", [128, NW], f32, kind="ExternalInput")
    sqdeg_in = nc.dram_tensor("sqdeg_own", [1, NPC], f32, kind="ExternalInput")
    Wmat = nc.dram_tensor("Wmat", [3, 64, 64], bf16, kind="ExternalInput")
    bias_in = nc.dram_tensor("bias_r", [3, 1, 64], f32, kind="ExternalInput")
    ident_in = nc.dram_tensor("ident", [128, 128], bf16, kind="ExternalInput")
    agents = nc.dram_tensor("agents_out", [3, NW * 32, 64], bf16,
                            kind="ExternalOutput")

    with tile.TileContext(nc) as tc:
        with (
            tc.tile_pool(name="const", bufs=1) as constp,
            tc.tile_pool(name="hT", bufs=2) as hTp,
            tc.tile_pool(name="xws", bufs=2) as xwsp,
            tc.tile_pool(name="acc", bufs=2) as accp,
            tc.tile_pool(name="msg", bufs=2) as msgp,
            tc.tile_pool(name="vm", bufs=2) as vmp,
            tc.tile_pool(name="small", bufs=4) as smallp,
            tc.tile_pool(name="ps_seg", bufs=3, space="PSUM") as ps_seg,
            tc.tile_pool(name="ps_xw", bufs=2, space="PSUM") as ps_xw,
            tc.tile_pool(name="ps_tr", bufs=2, space="PSUM") as ps_tr,
            tc.tile_pool(name="dram_ag", bufs=1, space="DRAM") as dram_ag,
            tc.tile_pool(name="dram_xw", bufs=1, space="DRAM") as dram_xw,
        ):
            meta_idx = constp.tile([128, T * 8], i16)
            nc.sync.dma_start(out=meta_idx[:], in_=src_idx[:, :])
            dinv_t = constp.tile([128, NW], f32)
            sqdeg_t = constp.tile([1, NPC], f32)
            ident_t = constp.tile([128, 128], bf16)
            nc.sync.dma_start(out=dinv_t[:], in_=dinv_in[:, :])
            nc.sync.dma_start(out=sqdeg_t[:], in_=sqdeg_in[:, :])
            nc.sync.dma_start(out=ident_t[:], in_=ident_in[:, :])
            w_tiles = []
            b_tiles = []
            for l in range(3):
                wt = constp.tile([64, 64], bf16, name=f"w{l}")
                bt = constp.tile([1, 64], f32, name=f"b{l}")
                nc.sync.dma_start(out=wt[:], in_=Wmat[l, :, :])
                nc.sync.dma_start(out=bt[:], in_=bias_in[l, :, :])
                w_tiles.append(wt)
                b_tiles.append(bt)

            hT_cur = hTp.tile([64, NPC], bf16, tag="hT", name="hT0")
            nc.sync.dma_start(out=hT_cur[:], in_=xT_own[:, :])

            for l in range(3):
                # ---- own-shard linear: xw = (h_own @ W_l) * dinv_own ----
                xw_stage = xwsp.tile([128, NW * 64], bf16, tag="xws",
                                     name=f"xws{l}")
                for w in range(NW):
                    ps = ps_xw.tile([128, 64], f32, tag="psxw",
                                    name=f"psxw{l}_{w}")
                    nc.tensor.matmul(
                        out=ps[:],
                        lhsT=hT_cur[:, w * 128:(w + 1) * 128],
                        rhs=w_tiles[l][:],
                        start=True, stop=True,
                    )
                    nc.scalar.mul(out=xw_stage[:, w * 64:(w + 1) * 64],
                                  in_=ps[:], mul=dinv_t[:, w:w + 1])

                ag_t = dram_ag.tile([NPC, 128], bf16, tag="ag", name=f"ag{l}")
                nc.sync.dma_start(
                    out=ag_t[:].rearrange("(w p) f -> p w f", p=128)[:, :, 0:64],
                    in_=xw_stage[:].rearrange("p (w f) -> p w f", f=64),
                )

                xwA = dram_xw.tile([ROWS_A, 128], bf16, tag="xwA",
                                   addr_space="Shared", name=f"xwA{l}")
                xwB = dram_xw.tile([ROWS_B, 128], bf16, tag="xwB",
                                   addr_space="Shared", name=f"xwB{l}")
                if "nocoll" in VAR:
                    nc.sync.dma_start(out=xwA[0:HALF_A, :],
                                      in_=ag_t[0:HALF_A, :])
                    nc.sync.dma_start(out=xwB[0:HALF_B, :],
                                      in_=ag_t[HALF_A:NPC, :])
                else:
                    nc.gpsimd.collective_compute(
                        "AllGather",
                        mybir.AluOpType.bypass,
                        replica_groups=[list(range(N_CORES))],
                        ins=[ag_t[0:HALF_A, :].opt()],
                        outs=[xwA.opt()],
                    )
                    nc.gpsimd.collective_compute(
                        "AllGather",
                        mybir.AluOpType.bypass,
                        replica_groups=[list(range(N_CORES))],
                        ins=[ag_t[HALF_A:NPC, :].opt()],
                        outs=[xwB.opt()],
                    )

                if l < 2:
                    hT_next = hTp.tile([64, NPC], bf16, tag="hT",
                                       name=f"hT{l + 1}")
                else:
                    hT_next = None

                # parked phase-A partial sums, one [128, 64] slice per window
                acc_t = accp.tile([128, NW * 64], bf16, tag="acc",
                                  name=f"acc{l}")

                def epilogue(w, cur_ps):
                    hwin = smallp.tile([128, 64], bf16, tag="hwin",
                                       name=f"hw{l}_{w}")
                    nc.scalar.activation(
                        out=hwin[:], in_=cur_ps[:],
                        func=mybir.ActivationFunctionType.Sigmoid,
                        scale=dinv_t[:, w:w + 1],
                    )
                    nc.sync.dma_start(
                        out=agents[l, w * 32:(w + 1) * 32, :],
                        in_=hwin[0:128:4, :],
                    )
                    if hT_next is not None:
                        pt = ps_tr.tile([64, 128], bf16, tag="tr",
                                        name=f"tr{l}_{w}")
                        nc.tensor.transpose(out=pt[:], in_=hwin[:],
                                            identity=ident_t[:])
                        nc.scalar.copy(
                            out=hT_next[:, w * 128:(w + 1) * 128],
                            in_=pt[:],
                        )

                # ---- gather + two-phase windowed segment-sum ----
                win_ps = {}
                for r, (t0, nt, h) in enumerate(runs):
                    msg = msgp.tile([128, max_run, 128], bf16, tag="msg",
                                    name=f"msg{l}_{r}")
                    vm_t = vmp.tile([128, max_run * 128], fp8, tag="vm",
                                    name=f"vm{l}_{r}")
                    nc.sync.dma_start(
                        out=vm_t[:, :nt * 128],
                        in_=vm_in[:, t0 * 128:(t0 + nt) * 128])
                    if "nogather" not in VAR:
                        nc.gpsimd.dma_gather(
                            out_ap=msg[:, :nt, :],
                            in_ap=(xwA[:] if h == 0 else xwB[:]),
                            idxs_ap=meta_idx[:, t0 * 8:(t0 + nt) * 8],
                            num_idxs=nt * 128,
                            num_idxs_reg=nt * 128,
                            elem_size=128,
                            single_packet=False,
                            queue_num=(r % nsq),
                        )
                    for j in range(nt):
                        t = t0 + j
                        w = int(tile_win[t])
                        if t == wfirst[h, w]:
                            cur = ps_seg.tile([128, 64], f32, tag="seg",
                                              name=f"seg{l}_{h}_{w}")
                            win_ps[w] = cur
                            if h == 0 or wfirst[0, w] < 0:
                                # open with bias: psum = sqrt(deg) x bias
                                nc.tensor.matmul(
                                    out=cur[:],
                                    lhsT=sqdeg_t[:, w * 128:(w + 1) * 128],
                                    rhs=b_tiles[l][:],
                                    start=True, stop=False,
                                )
                            else:
                                # re-inject parked phase-A sum
                                nc.tensor.matmul(
                                    out=cur[:],
                                    lhsT=ident_t[:],
                                    rhs=acc_t[:, w * 64:(w + 1) * 64],
                                    start=True, stop=False,
                                )
                        cur_ps = win_ps[w]
                        last = (t == wlast[h, w])
                        if "nomm" not in VAR:
                            nc.tensor.matmul(
                                out=cur_ps[:],
                                lhsT=vm_t[:, j * 128:(j + 1) * 128],
                                rhs=msg[:, j, 0:64],
                                start=False, stop=last,
                            )
                        elif last:
                            nc.scalar.copy(out=cur_ps[:], in_=cur_ps[:])
                        if last:
                            if h == 0 and wlast[1, w] >= 0:
                                # park phase-A sum in SBUF
                                nc.scalar.copy(
                                    out=acc_t[:, w * 64:(w + 1) * 64],
                                    in_=cur_ps[:])
                            else:
                                epilogue(w, cur_ps)

                # windows with no tiles at all (pad safety)
                for w in range(NW):
                    if wlast[0, w] < 0 and wlast[1, w] < 0:
                        cur = ps_seg.tile([128, 64], f32, tag="seg",
                                          name=f"segz{l}_{w}")
                        nc.tensor.matmul(
                            out=cur[:],
                            lhsT=sqdeg_t[:, w * 128:(w + 1) * 128],
                            rhs=b_tiles[l][:],
                            start=True, stop=True,
                        )
                        epilogue(w, cur)

                hT_cur = hT_next

    nc.compile()
    return nc


def kernel(**inputs):
    from concourse import bass_utils

    x = np.asarray(inputs["x"], dtype=np.float32)
    edge_index = np.asarray(inputs["edge_index"])
    agent_idx = np.asarray(inputs["agent_idx"], dtype=np.int64)
    Ws = [np.asarray(inputs[f"W{i}"], dtype=np.float32) for i in range(3)]
    bs = [np.asarray(inputs[f"b{i}"], dtype=np.float32) for i in range(3)]

    idx_arr, vm8, dinv_own, sqdeg_own, sched = _preprocess(edge_index)

    nc = _build_program(sched)

    xpad = np.zeros((N_CORES * NPC, D), np.float32)
    xpad[:N_NODES] = x
    Wstack = np.ascontiguousarray(
        np.stack(Ws)).astype(ml_dtypes.bfloat16)
    bias_stack = np.ascontiguousarray(
        np.stack([b[None, :] for b in bs])).astype(np.float32)
    ident = np.eye(128, dtype=ml_dtypes.bfloat16)

    in_maps = []
    for c in range(N_CORES):
        in_maps.append({
            "xT_own": np.ascontiguousarray(
                xpad[c * NPC:(c + 1) * NPC].T).astype(ml_dtypes.bfloat16),
            "src_idx": np.ascontiguousarray(idx_arr[c]),
            "vm8": np.ascontiguousarray(vm8[c]),
            "dinv_own": np.ascontiguousarray(dinv_own[c]),
            "sqdeg_own": np.ascontiguousarray(sqdeg_own[c]),
            "Wmat": Wstack,
            "bias_r": bias_stack,
            "ident": ident,
        })

    res = bass_utils.run_bass_kernel_spmd(
        nc, in_maps, core_ids=list(range(N_CORES)))

    taps = np.stack([np.asarray(res.results[c]["agents_out"])
                     .astype(np.float32) for c in range(N_CORES)])
    # taps[c, l, r, :] = h_l for node (c*NPC + 4*r)
    n_agents = agent_idx.shape[0]
    out = np.empty((n_agents, 3 * D), np.float32)
    c_of = agent_idx // NPC
    r_of = (agent_idx % NPC) // 4
    for l in range(3):
        out[:, l * D:(l + 1) * D] = taps[c_of, l, r_of, :]
    return out


# revision 21
# speedup vs baseline: 1.9772x; 1.4941x over previous
"""3-layer GCN (PyG GCNConv-style) on 8 Trainium2 NeuronCores — v2.

Strategy (graph/data parallel; nodes sharded by destination core):
  - Nodes partitioned contiguously: 6272 per core (49 windows x 128). Edges
    (incl. host-added self-loops) are owned by the core owning their dst.
  - Normalization is folded away: the gathered xw table holds
    dinv[src] * (h @ W) rows, and the window epilogue applies the dinv[dst]
    factor as the per-partition `scale` of the sigmoid activation. The bias
    is injected into PSUM via a K=1 rank-1 matmul with a sqrt(deg[dst])
    column so it survives the later dinv[dst] scaling.
  - The per-edge one-hot scatter matrices are graph-static: built ONCE on
    the host in fp8 (exact for 0/1) and streamed from DRAM each layer,
    freeing the Vector engine entirely (v1 spent 85% of the span there).
  - Gathers use int16 indices, so the 50176-row xw table is split into
    half-tables A (local row < 3200, 25600 rows) and B (24576 rows). Each
    layer runs two phases: phase A processes every window's A-half tiles
    (window-major, one live PSUM bank, parked to SBUF in bf16 at window
    close), phase B re-injects the parked sum via an identity matmul and
    finishes the window (sigmoid epilogue, agent tap, PE transpose into the
    next layer's hT). Phases are split into ~128-tile gather chunks — the
    per-call Q7 descriptor-gen cost is ~20us flat, so calls are few and big.
  - Per layer: 49 own-shard bf16 matmuls -> ScalarE evac (x dinv, cast bf16)
    -> one DMA into the padded [6272, 128]-bf16 shard -> two AllGathers
    (A-half first so phase-A gathers start sooner).

Host-side work: graph preprocessing (degrees, edge layout, one-hot tiles)
and final output assembly.
"""

import sys

sys.path.insert(0, "/opt/trn_rl_repo")

import numpy as np
import ml_dtypes

N_NODES = 50000
D = 64
N_CORES = 8
WSZ = 128               # dst-window size (PSUM partition dim)
NW = 49                 # windows per core
NPC = NW * WSZ          # 6272 padded nodes per core (50176 total >= 50000)
HALF_A = 3200           # local rows < HALF_A -> table A (25 windows' rows)
HALF_B = NPC - HALF_A   # 3072 rows -> table B
ROWS_A = N_CORES * HALF_A   # 25600 (< 32767, int16-addressable)
ROWS_B = N_CORES * HALF_B   # 24576
CMAX = 64               # max tiles per gather chunk; small chunks let 4
                        # gathers (one per SWDGE queue = Q7 core pair) overlap


def _preprocess(edge_index):
    """Edge layout + one-hot scatter tiles.

    Tile stream: [phase A: w0..w48, each window's A-half tiles]
                 [phase B: w0..w48, each window's B-half tiles].
    SPMD: tile counts per (window, half) are maxed over cores; padded slots
    get all-zero one-hot rows so they contribute nothing.
    """
    src = np.asarray(edge_index[0], dtype=np.int64)
    dst = np.asarray(edge_index[1], dtype=np.int64)

    deg = np.bincount(dst, minlength=N_NODES).astype(np.float32) + 1.0
    dinv = (1.0 / np.sqrt(deg)).astype(np.float32)
    sqdeg = np.sqrt(deg).astype(np.float32)

    # self-loops are NOT gathered: their dinv[i]*xw[i] rows live in the
    # local xw_stage and are added via one identity matmul per window
    s_all = src
    d_all = dst

    core = d_all // NPC
    local = d_all - core * NPC
    win = local // WSZ
    col = local % WSZ

    s_core = s_all // NPC
    s_loc = s_all - s_core * NPC
    half = (s_loc >= HALF_A).astype(np.int64)
    idx16 = np.where(half == 0, s_core * HALF_A + s_loc,
                     s_core * HALF_B + (s_loc - HALF_A))

    # group edges by (core, half, win)
    key = (core * 2 + half) * NW + win
    nkey = N_CORES * 2 * NW
    order = np.argsort(key, kind="stable")
    key_sorted = key[order]
    bounds = np.searchsorted(key_sorted, np.arange(nkey + 1))
    cnt = (bounds[1:] - bounds[:-1]).reshape(N_CORES, 2, NW)

    # uniform tiles per (half, win), maxed over cores
    n_th = -(-cnt.max(axis=0) // WSZ)               # [2, NW]
    T = int(n_th.sum())

    # tile stream + gather chunks (runs)
    tile_win = []
    runs = []                                       # (t0, nt, half)
    win_tile_base = np.zeros((2, NW), np.int64)
    for h in (0, 1):
        p0 = len(tile_win)
        for w in range(NW):
            win_tile_base[h, w] = len(tile_win)
            tile_win += [w] * int(n_th[h, w])
        np_h = len(tile_win) - p0                   # tiles in this phase
        if np_h == 0:
            continue
        n_chunks = -(-np_h // CMAX)
        splits = np.linspace(p0, p0 + np_h, n_chunks + 1).astype(np.int64)
        for a, b in zip(splits[:-1], splits[1:]):
            if b > a:
                runs.append((int(a), int(b - a), h))
    tile_win = np.asarray(tile_win)
    assert len(tile_win) == T
    max_run = max(nt for _, nt, _ in runs)

    # per-window first/last tile within each phase (-1 if none)
    wfirst = np.full((2, NW), -1, np.int64)
    wlast = np.full((2, NW), -1, np.int64)
    for h in (0, 1):
        for w in range(NW):
            if n_th[h, w] > 0:
                wfirst[h, w] = win_tile_base[h, w]
                wlast[h, w] = win_tile_base[h, w] + n_th[h, w] - 1

    # per-core edge slot arrays
    idx_flat = np.zeros((N_CORES, T * WSZ), np.int16)
    vm8 = np.zeros((N_CORES, WSZ, T * WSZ), ml_dtypes.float8_e4m3)
    for c in range(N_CORES):
        for h in (0, 1):
            for w in range(NW):
                gidx = (c * 2 + h) * NW + w
                e0, e1 = bounds[gidx], bounds[gidx + 1]
                n = e1 - e0
                if n == 0:
                    continue
                sel = order[e0:e1]
                base = win_tile_base[h, w] * WSZ
                pos = base + np.arange(n)
                idx_flat[c, pos] = idx16[sel].astype(np.int16)
                tt = pos // WSZ
                pp = pos % WSZ
                vm8[c, pp, tt * WSZ + col[sel]] = 1.0

    # wrap indices for dma_gather: [128, T*8] int16,
    # arr[p, t*8 + cc] = idx[t*128 + cc*16 + (p % 16)]
    w16 = idx_flat.reshape(N_CORES, T, 8, 16).transpose(0, 3, 1, 2).reshape(
        N_CORES, 16, T * 8)
    idx_arr = np.tile(w16, (1, 8, 1))               # [N_CORES, 128, T*8]

    # per-core epilogue scale layouts
    dinv_pad = np.ones(N_CORES * NPC, np.float32)
    sqdeg_pad = np.ones(N_CORES * NPC, np.float32)
    dinv_pad[:N_NODES] = dinv
    sqdeg_pad[:N_NODES] = sqdeg
    dinv_own = dinv_pad.reshape(N_CORES, NW, WSZ).transpose(0, 2, 1).copy()
    sqdeg_own = sqdeg_pad.reshape(N_CORES, 1, NPC).astype(ml_dtypes.bfloat16)

    sched = dict(T=T, runs=runs, tile_win=tile_win, n_th=n_th,
                 wfirst=wfirst, wlast=wlast, max_run=max_run)
    return idx_arr, vm8, dinv_own, sqdeg_own, sched


def _build_program(sched):
    import os
    VAR = set(os.environ.get("KVAR", "").split(","))
    import concourse.bass as bass
    import concourse.bacc as bacc
    import concourse.tile as tile
    from concourse import mybir

    f32 = mybir.dt.float32
    bf16 = mybir.dt.bfloat16
    fp8 = mybir.dt.float8e4
    i16 = mybir.dt.int16

    T = sched["T"]
    runs = sched["runs"]
    tile_win = sched["tile_win"]
    n_th = sched["n_th"]
    wfirst = sched["wfirst"]
    wlast = sched["wlast"]
    max_run = sched["max_run"]

    nsq = 4
    nc = bacc.Bacc("TRN2", target_bir_lowering=False, debug=False,
                   num_devices=N_CORES, num_swdge_queues=nsq)

    xT_own = nc.dram_tensor("xT_own", [64, NPC], bf16, kind="ExternalInput")
    src_idx = nc.dram_tensor("src_idx", [128, T * 8], i16, kind="ExternalInput")
    vm_in = nc.dram_tensor("vm8", [128, T * 128], fp8, kind="ExternalInput")
    dinv_in = nc.dram_tensor("dinv_own", [128, NW], f32, kind="ExternalInput")
    sqdeg_in = nc.dram_tensor("sqdeg_own", [1, NPC], bf16, kind="ExternalInput")
    Wmat = nc.dram_tensor("Wmat", [3, 64, 64], bf16, kind="ExternalInput")
    bias_in = nc.dram_tensor("bias_r", [3, 1, 64], bf16, kind="ExternalInput")
    ident_in = nc.dram_tensor("ident", [128, 128], bf16, kind="ExternalInput")
    agents = nc.dram_tensor("agents_out", [3, NW * 32, 64], bf16,
                            kind="ExternalOutput")

    with tile.TileContext(nc) as tc:
        with (
            tc.tile_pool(name="const", bufs=1) as constp,
            tc.tile_pool(name="hT", bufs=2) as hTp,
            tc.tile_pool(name="xws", bufs=2) as xwsp,
            tc.tile_pool(name="acc", bufs=2) as accp,
            tc.tile_pool(name="msg", bufs=4) as msgp,
            tc.tile_pool(name="vm", bufs=4) as vmp,
            tc.tile_pool(name="small", bufs=4) as smallp,
            tc.tile_pool(name="ps_seg", bufs=3, space="PSUM") as ps_seg,
            tc.tile_pool(name="ps_xw", bufs=2, space="PSUM") as ps_xw,
            tc.tile_pool(name="ps_tr", bufs=2, space="PSUM") as ps_tr,
            tc.tile_pool(name="dram_ag", bufs=1, space="DRAM") as dram_ag,
            tc.tile_pool(name="dram_xw", bufs=1, space="DRAM") as dram_xw,
        ):
            meta_idx = constp.tile([128, T * 8], i16)
            nc.sync.dma_start(out=meta_idx[:], in_=src_idx[:, :])
            dinv_t = constp.tile([128, NW], f32)
            sqdeg_t = constp.tile([1, NPC], bf16)
            ident_t = constp.tile([128, 128], bf16)
            nc.sync.dma_start(out=dinv_t[:], in_=dinv_in[:, :])
            nc.sync.dma_start(out=sqdeg_t[:], in_=sqdeg_in[:, :])
            nc.sync.dma_start(out=ident_t[:], in_=ident_in[:, :])
            w_tiles = []
            b_tiles = []
            for l in range(3):
                wt = constp.tile([64, 64], bf16, name=f"w{l}")
                bt = constp.tile([1, 64], bf16, name=f"b{l}")
                nc.sync.dma_start(out=wt[:], in_=Wmat[l, :, :])
                nc.sync.dma_start(out=bt[:], in_=bias_in[l, :, :])
                w_tiles.append(wt)
                b_tiles.append(bt)

            hT_cur = hTp.tile([64, NPC], bf16, tag="hT", name="hT0")
            nc.sync.dma_start(out=hT_cur[:], in_=xT_own[:, :])

            grun = 0    # global run counter: keeps SWDGE queue aligned with
                        # the msg/vm pools' round-robin buffer assignment
            for l in range(3):
                # ---- own-shard linear: xw = (h_own @ W_l) * dinv_own ----
                xw_stage = xwsp.tile([128, NW * 64], bf16, tag="xws",
                                     name=f"xws{l}")
                for w in range(NW):
                    ps = ps_xw.tile([128, 64], f32, tag="psxw",
                                    name=f"psxw{l}_{w}")
                    nc.tensor.matmul(
                        out=ps[:],
                        lhsT=hT_cur[:, w * 128:(w + 1) * 128],
                        rhs=w_tiles[l][:],
                        start=True, stop=True,
                    )
                    nc.scalar.mul(out=xw_stage[:, w * 64:(w + 1) * 64],
                                  in_=ps[:], mul=dinv_t[:, w:w + 1])

                ag_t = dram_ag.tile([NPC, 128], bf16, tag="ag", name=f"ag{l}")
                nc.sync.dma_start(
                    out=ag_t[:].rearrange("(w p) f -> p w f", p=128)[:, :, 0:64],
                    in_=xw_stage[:].rearrange("p (w f) -> p w f", f=64),
                )

                xwA = dram_xw.tile([ROWS_A, 128], bf16, tag="xwA",
                                   addr_space="Shared", name=f"xwA{l}")
                xwB = dram_xw.tile([ROWS_B, 128], bf16, tag="xwB",
                                   addr_space="Shared", name=f"xwB{l}")
                if "nocoll" in VAR:
                    nc.sync.dma_start(out=xwA[0:HALF_A, :],
                                      in_=ag_t[0:HALF_A, :])
                    nc.sync.dma_start(out=xwB[0:HALF_B, :],
                                      in_=ag_t[HALF_A:NPC, :])
                else:
                    nc.gpsimd.collective_compute(
                        "AllGather",
                        mybir.AluOpType.bypass,
                        replica_groups=[list(range(N_CORES))],
                        ins=[ag_t[0:HALF_A, :].opt()],
                        outs=[xwA.opt()],
                    )
                    nc.gpsimd.collective_compute(
                        "AllGather",
                        mybir.AluOpType.bypass,
                        replica_groups=[list(range(N_CORES))],
                        ins=[ag_t[HALF_A:NPC, :].opt()],
                        outs=[xwB.opt()],
                    )

                if l < 2:
                    hT_next = hTp.tile([64, NPC], bf16, tag="hT",
                                       name=f"hT{l + 1}")
                else:
                    hT_next = None

                # parked phase-A partial sums, one [128, 64] slice per window
                acc_t = accp.tile([128, NW * 64], bf16, tag="acc",
                                  name=f"acc{l}")

                def epilogue(w, cur_ps):
                    hwin = smallp.tile([128, 64], bf16, tag="hwin",
                                       name=f"hw{l}_{w}")
                    nc.scalar.activation(
                        out=hwin[:], in_=cur_ps[:],
                        func=mybir.ActivationFunctionType.Sigmoid,
                        scale=dinv_t[:, w:w + 1],
                    )
                    nc.sync.dma_start(
                        out=agents[l, w * 32:(w + 1) * 32, :],
                        in_=hwin[0:128:4, :],
                    )
                    if hT_next is not None:
                        pt = ps_tr.tile([64, 128], bf16, tag="tr",
                                        name=f"tr{l}_{w}")
                        nc.tensor.transpose(out=pt[:], in_=hwin[:],
                                            identity=ident_t[:])
                        nc.scalar.copy(
                            out=hT_next[:, w * 128:(w + 1) * 128],
                            in_=pt[:],
                        )

                # ---- gather + two-phase windowed segment-sum ----
                win_ps = {}
                for r, (t0, nt, h) in enumerate(runs):
                    msg = msgp.tile([128, max_run, 128], bf16, tag="msg",
                                    name=f"msg{l}_{r}")
                    vm_t = vmp.tile([128, max_run * 128], fp8, tag="vm",
                                    name=f"vm{l}_{r}")
                    nc.sync.dma_start(
                        out=vm_t[:, :nt * 128],
                        in_=vm_in[:, t0 * 128:(t0 + nt) * 128])
                    if "nogather" not in VAR:
                        nc.gpsimd.dma_gather(
                            out_ap=msg[:, :nt, :],
                            in_ap=(xwA[:] if h == 0 else xwB[:]),
                            idxs_ap=meta_idx[:, t0 * 8:(t0 + nt) * 8],
                            num_idxs=nt * 128,
                            num_idxs_reg=nt * 128,
                            elem_size=128,
                            single_packet=False,
                            queue_num=(grun % nsq),
                        )
                    grun += 1
                    for j in range(nt):
                        t = t0 + j
                        w = int(tile_win[t])
                        if t == wfirst[h, w]:
                            cur = ps_seg.tile([128, 64], f32, tag="seg",
                                              name=f"seg{l}_{h}_{w}")
                            win_ps[w] = cur
                            if h == 0 or wfirst[0, w] < 0:
                                # open with bias: psum = sqrt(deg) x bias
                                nc.tensor.matmul(
                                    out=cur[:],
                                    lhsT=sqdeg_t[:, w * 128:(w + 1) * 128],
                                    rhs=b_tiles[l][:],
                                    start=True, stop=False,
                                )
                                # self-loop term: += dinv[i] * xw[i]
                                nc.tensor.matmul(
                                    out=cur[:],
                                    lhsT=ident_t[:],
                                    rhs=xw_stage[:, w * 64:(w + 1) * 64],
                                    start=False, stop=False,
                                )
                            else:
                                # re-inject parked phase-A sum
                                nc.tensor.matmul(
                                    out=cur[:],
                                    lhsT=ident_t[:],
                                    rhs=acc_t[:, w * 64:(w + 1) * 64],
                                    start=True, stop=False,
                                )
                        cur_ps = win_ps[w]
                        last = (t == wlast[h, w])
                        if "nomm" not in VAR:
                            nc.tensor.matmul(
                                out=cur_ps[:],
                                lhsT=vm_t[:, j * 128:(j + 1) * 128],
                                rhs=msg[:, j, 0:64],
                                start=False, stop=last,
                            )
                        elif last:
                            nc.scalar.copy(out=cur_ps[:], in_=cur_ps[:])
                        if last:
                            if h == 0 and wlast[1, w] >= 0:
                                # park phase-A sum in SBUF
                                nc.scalar.copy(
                                    out=acc_t[:, w * 64:(w + 1) * 64],
                                    in_=cur_ps[:])
                            else:
                                epilogue(w, cur_ps)

                # windows with no tiles at all (pad safety)
                for w in range(NW):
                    if wlast[0, w] < 0 and wlast[1, w] < 0:
                        cur = ps_seg.tile([128, 64], f32, tag="seg",
                                          name=f"segz{l}_{w}")
                        nc.tensor.matmul(
                            out=cur[:],
                            lhsT=sqdeg_t[:, w * 128:(w + 1) * 128],
                            rhs=b_tiles[l][:],
                            start=True, stop=False,
                        )
                        nc.tensor.matmul(
                            out=cur[:],
                            lhsT=ident_t[:],
                            rhs=xw_stage[:, w * 64:(w + 1) * 64],
                            start=False, stop=True,
                        )
                        epilogue(w, cur)

                hT_cur = hT_next

    nc.compile()
    return nc


def kernel(**inputs):
    from concourse import bass_utils

    x = np.asarray(inputs["x"], dtype=np.float32)
    edge_index = np.asarray(inputs["edge_index"])
    agent_idx = np.asarray(inputs["agent_idx"], dtype=np.int64)
    Ws = [np.asarray(inputs[f"W{i}"], dtype=np.float32) for i in range(3)]
    bs = [np.asarray(inputs[f"b{i}"], dtype=np.float32) for i in range(3)]

    idx_arr, vm8, dinv_own, sqdeg_own, sched = _preprocess(edge_index)

    nc = _build_program(sched)

    xpad = np.zeros((N_CORES * NPC, D), np.float32)
    xpad[:N_NODES] = x
    Wstack = np.ascontiguousarray(
        np.stack(Ws)).astype(ml_dtypes.bfloat16)
    bias_stack = np.ascontiguousarray(
        np.stack([b[None, :] for b in bs])).astype(ml_dtypes.bfloat16)
    ident = np.eye(128, dtype=ml_dtypes.bfloat16)

    in_maps = []
    for c in range(N_CORES):
        in_maps.append({
            "xT_own": np.ascontiguousarray(
                xpad[c * NPC:(c + 1) * NPC].T).astype(ml_dtypes.bfloat16),
            "src_idx": np.ascontiguousarray(idx_arr[c]),
            "vm8": np.ascontiguousarray(vm8[c]),
            "dinv_own": np.ascontiguousarray(dinv_own[c]),
            "sqdeg_own": np.ascontiguousarray(sqdeg_own[c]),
            "Wmat": Wstack,
            "bias_r": bias_stack,
            "ident": ident,
        })

    res = bass_utils.run_bass_kernel_spmd(
        nc, in_maps, core_ids=list(range(N_CORES)))

    taps = np.stack([np.asarray(res.results[c]["agents_out"])
                     .astype(np.float32) for c in range(N_CORES)])
    # taps[c, l, r, :] = h_l for node (c*NPC + 4*r)
    n_agents = agent_idx.shape[0]
    out = np.empty((n_agents, 3 * D), np.float32)
    c_of = agent_idx // NPC
    r_of = (agent_idx % NPC) // 4
    for l in range(3):
        out[:, l * D:(l + 1) * D] = taps[c_of, l, r_of, :]
    return out


# revision 29
# speedup vs baseline: 2.4793x; 1.2540x over previous
"""3-layer GCN (PyG GCNConv-style) on 8 Trainium2 NeuronCores — v2.

Strategy (graph/data parallel; nodes sharded by destination core):
  - Nodes partitioned contiguously: 6272 per core (49 windows x 128). Edges
    (incl. host-added self-loops) are owned by the core owning their dst.
  - Normalization is folded away: the gathered xw table holds
    dinv[src] * (h @ W) rows, and the window epilogue applies the dinv[dst]
    factor as the per-partition `scale` of the sigmoid activation. The bias
    is injected into PSUM via a K=1 rank-1 matmul with a sqrt(deg[dst])
    column so it survives the later dinv[dst] scaling.
  - The per-edge one-hot scatter matrices are graph-static: built ONCE on
    the host in fp8 (exact for 0/1) and streamed from DRAM each layer,
    freeing the Vector engine entirely (v1 spent 85% of the span there).
  - Gathers use int16 indices, so the 50176-row xw table is split into
    half-tables A (local row < 3200, 25600 rows) and B (24576 rows). Each
    layer runs two phases: phase A processes every window's A-half tiles
    (window-major, one live PSUM bank, parked to SBUF in bf16 at window
    close), phase B re-injects the parked sum via an identity matmul and
    finishes the window (sigmoid epilogue, agent tap, PE transpose into the
    next layer's hT). Phases are split into CMAX-tile gather chunks, each
    issued as 8-tile sub-gathers (single_packet coalescing caps a call at
    64 descs/engine) spread round-robin over the 4 SWDGE queues so all
    four Q7 core pairs generate descriptors in parallel (~7.4ns/index each).
  - Per layer: 49 own-shard bf16 matmuls -> ScalarE evac (x dinv, cast bf16)
    -> one DMA into the padded [6272, 128]-bf16 shard -> two AllGathers
    (A-half first so phase-A gathers start sooner).

Host-side work: graph preprocessing (degrees, edge layout, one-hot tiles)
and final output assembly.
"""

import sys

sys.path.insert(0, "/opt/trn_rl_repo")

import numpy as np
import ml_dtypes

N_NODES = 50000
D = 64
N_CORES = 8
WSZ = 128               # dst-window size (PSUM partition dim)
NW = 49                 # windows per core
NPC = NW * WSZ          # 6272 padded nodes per core (50176 total >= 50000)
HALF_A = 3200           # local rows < HALF_A -> table A (25 windows' rows)
HALF_B = NPC - HALF_A   # 3072 rows -> table B
ROWS_A = N_CORES * HALF_A   # 25600 (< 32767, int16-addressable)
ROWS_B = N_CORES * HALF_B   # 24576
CMAX = 48               # max tiles per gather chunk; small chunks + deep
                        # buffering let 4 queues (= Q7 core pairs) gen in parallel


def _preprocess(edge_index):
    """Edge layout + one-hot scatter tiles.

    Tile stream: [phase A: w0..w48, each window's A-half tiles]
                 [phase B: w0..w48, each window's B-half tiles].
    SPMD: tile counts per (window, half) are maxed over cores; padded slots
    get all-zero one-hot rows so they contribute nothing.
    """
    src = np.asarray(edge_index[0], dtype=np.int64)
    dst = np.asarray(edge_index[1], dtype=np.int64)

    deg = np.bincount(dst, minlength=N_NODES).astype(np.float32) + 1.0
    dinv = (1.0 / np.sqrt(deg)).astype(np.float32)
    sqdeg = np.sqrt(deg).astype(np.float32)

    # self-loops are NOT gathered: their dinv[i]*xw[i] rows live in the
    # local xw_stage and are added via one identity matmul per window
    s_all = src
    d_all = dst

    core = d_all // NPC
    local = d_all - core * NPC
    win = local // WSZ
    col = local % WSZ

    s_core = s_all // NPC
    s_loc = s_all - s_core * NPC
    half = (s_loc >= HALF_A).astype(np.int64)
    idx16 = np.where(half == 0, s_core * HALF_A + s_loc,
                     s_core * HALF_B + (s_loc - HALF_A))

    # group edges by (core, half, win)
    key = (core * 2 + half) * NW + win
    nkey = N_CORES * 2 * NW
    order = np.argsort(key, kind="stable")
    key_sorted = key[order]
    bounds = np.searchsorted(key_sorted, np.arange(nkey + 1))
    cnt = (bounds[1:] - bounds[:-1]).reshape(N_CORES, 2, NW)

    # uniform tiles per (half, win), maxed over cores
    n_th = -(-cnt.max(axis=0) // WSZ)               # [2, NW]
    T = int(n_th.sum())

    # tile stream + gather chunks (runs)
    tile_win = []
    runs = []                                       # (t0, nt, half)
    win_tile_base = np.zeros((2, NW), np.int64)
    for h in (0, 1):
        p0 = len(tile_win)
        for w in range(NW):
            win_tile_base[h, w] = len(tile_win)
            tile_win += [w] * int(n_th[h, w])
        np_h = len(tile_win) - p0                   # tiles in this phase
        if np_h == 0:
            continue
        n_chunks = -(-np_h // CMAX)
        splits = np.linspace(p0, p0 + np_h, n_chunks + 1).astype(np.int64)
        for a, b in zip(splits[:-1], splits[1:]):
            if b > a:
                runs.append((int(a), int(b - a), h))
    tile_win = np.asarray(tile_win)
    assert len(tile_win) == T
    max_run = max(nt for _, nt, _ in runs)

    # per-window first/last tile within each phase (-1 if none)
    wfirst = np.full((2, NW), -1, np.int64)
    wlast = np.full((2, NW), -1, np.int64)
    for h in (0, 1):
        for w in range(NW):
            if n_th[h, w] > 0:
                wfirst[h, w] = win_tile_base[h, w]
                wlast[h, w] = win_tile_base[h, w] + n_th[h, w] - 1

    # per-core edge slot arrays
    idx_flat = np.zeros((N_CORES, T * WSZ), np.int16)
    vm8 = np.zeros((N_CORES, WSZ, T * WSZ), ml_dtypes.float8_e4m3)
    for c in range(N_CORES):
        for h in (0, 1):
            for w in range(NW):
                gidx = (c * 2 + h) * NW + w
                e0, e1 = bounds[gidx], bounds[gidx + 1]
                n = e1 - e0
                if n == 0:
                    continue
                sel = order[e0:e1]
                base = win_tile_base[h, w] * WSZ
                pos = base + np.arange(n)
                idx_flat[c, pos] = idx16[sel].astype(np.int16)
                tt = pos // WSZ
                pp = pos % WSZ
                vm8[c, pp, tt * WSZ + col[sel]] = 1.0

    # wrap indices for dma_gather: [128, T*8] int16,
    # arr[p, t*8 + cc] = idx[t*128 + cc*16 + (p % 16)]
    w16 = idx_flat.reshape(N_CORES, T, 8, 16).transpose(0, 3, 1, 2).reshape(
        N_CORES, 16, T * 8)
    idx_arr = np.tile(w16, (1, 8, 1))               # [N_CORES, 128, T*8]

    # per-core epilogue scale layouts
    dinv_pad = np.ones(N_CORES * NPC, np.float32)
    sqdeg_pad = np.ones(N_CORES * NPC, np.float32)
    dinv_pad[:N_NODES] = dinv
    sqdeg_pad[:N_NODES] = sqdeg
    dinv_own = dinv_pad.reshape(N_CORES, NW, WSZ).transpose(0, 2, 1).copy()
    sqdeg_own = sqdeg_pad.reshape(N_CORES, 1, NPC).astype(ml_dtypes.bfloat16)

    sched = dict(T=T, runs=runs, tile_win=tile_win, n_th=n_th,
                 wfirst=wfirst, wlast=wlast, max_run=max_run)
    return idx_arr, vm8, dinv_own, sqdeg_own, sched


def _build_program(sched):
    import os
    VAR = set(os.environ.get("KVAR", "").split(","))
    import concourse.bass as bass
    import concourse.bacc as bacc
    import concourse.tile as tile
    from concourse import mybir

    f32 = mybir.dt.float32
    bf16 = mybir.dt.bfloat16
    fp8 = mybir.dt.float8e4
    i16 = mybir.dt.int16

    T = sched["T"]
    runs = sched["runs"]
    tile_win = sched["tile_win"]
    n_th = sched["n_th"]
    wfirst = sched["wfirst"]
    wlast = sched["wlast"]
    max_run = sched["max_run"]

    nsq = 4
    nc = bacc.Bacc("TRN2", target_bir_lowering=False, debug=False,
                   num_devices=N_CORES, num_swdge_queues=nsq)

    xT_own = nc.dram_tensor("xT_own", [64, NPC], bf16, kind="ExternalInput")
    src_idx = nc.dram_tensor("src_idx", [128, T * 8], i16, kind="ExternalInput")
    vm_in = nc.dram_tensor("vm8", [128, T * 128], fp8, kind="ExternalInput")
    dinv_in = nc.dram_tensor("dinv_own", [128, NW], f32, kind="ExternalInput")
    sqdeg_in = nc.dram_tensor("sqdeg_own", [1, NPC], bf16, kind="ExternalInput")
    Wmat = nc.dram_tensor("Wmat", [3, 64, 64], bf16, kind="ExternalInput")
    bias_in = nc.dram_tensor("bias_r", [3, 1, 64], bf16, kind="ExternalInput")
    ident_in = nc.dram_tensor("ident", [128, 128], bf16, kind="ExternalInput")
    agents = nc.dram_tensor("agents_out", [3, NW * 32, 64], bf16,
                            kind="ExternalOutput")

    with tile.TileContext(nc) as tc:
        with (
            tc.tile_pool(name="const", bufs=1) as constp,
            tc.tile_pool(name="hT", bufs=2) as hTp,
            tc.tile_pool(name="xws", bufs=2) as xwsp,
            tc.tile_pool(name="acc", bufs=2) as accp,
            tc.tile_pool(name="msg", bufs=6) as msgp,
            tc.tile_pool(name="vm", bufs=6) as vmp,
            tc.tile_pool(name="small", bufs=4) as smallp,
            tc.tile_pool(name="ps_seg", bufs=3, space="PSUM") as ps_seg,
            tc.tile_pool(name="ps_xw", bufs=2, space="PSUM") as ps_xw,
            tc.tile_pool(name="ps_tr", bufs=2, space="PSUM") as ps_tr,
            tc.tile_pool(name="dram_ag", bufs=1, space="DRAM") as dram_ag,
            tc.tile_pool(name="dram_xw", bufs=1, space="DRAM") as dram_xw,
        ):
            meta_idx = constp.tile([128, T * 8], i16)
            nc.sync.dma_start(out=meta_idx[:], in_=src_idx[:, :])
            dinv_t = constp.tile([128, NW], f32)
            sqdeg_t = constp.tile([1, NPC], bf16)
            ident_t = constp.tile([128, 128], bf16)
            nc.sync.dma_start(out=dinv_t[:], in_=dinv_in[:, :])
            nc.sync.dma_start(out=sqdeg_t[:], in_=sqdeg_in[:, :])
            nc.sync.dma_start(out=ident_t[:], in_=ident_in[:, :])
            w_tiles = []
            b_tiles = []
            for l in range(3):
                wt = constp.tile([64, 64], bf16, name=f"w{l}")
                bt = constp.tile([1, 64], bf16, name=f"b{l}")
                nc.sync.dma_start(out=wt[:], in_=Wmat[l, :, :])
                nc.sync.dma_start(out=bt[:], in_=bias_in[l, :, :])
                w_tiles.append(wt)
                b_tiles.append(bt)

            hT_cur = hTp.tile([64, NPC], bf16, tag="hT", name="hT0")
            nc.sync.dma_start(out=hT_cur[:], in_=xT_own[:, :])

            gg = 0      # global gather counter: Tile assigns SWDGE sems as
                        # gather#%8, so queue gather#%4 keeps sem<->queue 1:1
            for l in range(3):
                # ---- own-shard linear: xw = (h_own @ W_l) * dinv_own ----
                xw_stage = xwsp.tile([128, NW * 64], bf16, tag="xws",
                                     name=f"xws{l}")
                for w in range(NW):
                    ps = ps_xw.tile([128, 64], f32, tag="psxw",
                                    name=f"psxw{l}_{w}")
                    nc.tensor.matmul(
                        out=ps[:],
                        lhsT=hT_cur[:, w * 128:(w + 1) * 128],
                        rhs=w_tiles[l][:],
                        start=True, stop=True,
                    )
                    nc.scalar.mul(out=xw_stage[:, w * 64:(w + 1) * 64],
                                  in_=ps[:], mul=dinv_t[:, w:w + 1])

                ag_t = dram_ag.tile([NPC, 128], bf16, tag="ag", name=f"ag{l}")
                nc.sync.dma_start(
                    out=ag_t[:].rearrange("(w p) f -> p w f", p=128)[:, :, 0:64],
                    in_=xw_stage[:].rearrange("p (w f) -> p w f", f=64),
                )

                xwA = dram_xw.tile([ROWS_A, 128], bf16, tag="xwA",
                                   addr_space="Shared", name=f"xwA{l}")
                xwB = dram_xw.tile([ROWS_B, 128], bf16, tag="xwB",
                                   addr_space="Shared", name=f"xwB{l}")
                if "nocoll" in VAR:
                    nc.sync.dma_start(out=xwA[0:HALF_A, :],
                                      in_=ag_t[0:HALF_A, :])
                    nc.sync.dma_start(out=xwB[0:HALF_B, :],
                                      in_=ag_t[HALF_A:NPC, :])
                else:
                    nc.gpsimd.collective_compute(
                        "AllGather",
                        mybir.AluOpType.bypass,
                        replica_groups=[list(range(N_CORES))],
                        ins=[ag_t[0:HALF_A, :].opt()],
                        outs=[xwA.opt()],
                    )
                    nc.gpsimd.collective_compute(
                        "AllGather",
                        mybir.AluOpType.bypass,
                        replica_groups=[list(range(N_CORES))],
                        ins=[ag_t[HALF_A:NPC, :].opt()],
                        outs=[xwB.opt()],
                    )

                if l < 2:
                    hT_next = hTp.tile([64, NPC], bf16, tag="hT",
                                       name=f"hT{l + 1}")
                else:
                    hT_next = None

                # parked phase-A partial sums, one [128, 64] slice per window
                acc_t = accp.tile([128, NW * 64], bf16, tag="acc",
                                  name=f"acc{l}")

                def epilogue(w, cur_ps):
                    hwin = smallp.tile([128, 64], bf16, tag="hwin",
                                       name=f"hw{l}_{w}")
                    nc.scalar.activation(
                        out=hwin[:], in_=cur_ps[:],
                        func=mybir.ActivationFunctionType.Sigmoid,
                        scale=dinv_t[:, w:w + 1],
                    )
                    nc.sync.dma_start(
                        out=agents[l, w * 32:(w + 1) * 32, :],
                        in_=hwin[0:128:4, :],
                    )
                    if hT_next is not None:
                        pt = ps_tr.tile([64, 128], bf16, tag="tr",
                                        name=f"tr{l}_{w}")
                        nc.tensor.transpose(out=pt[:], in_=hwin[:],
                                            identity=ident_t[:])
                        nc.scalar.copy(
                            out=hT_next[:, w * 128:(w + 1) * 128],
                            in_=pt[:],
                        )

                # ---- gather + two-phase windowed segment-sum ----
                win_ps = {}
                for r, (t0, nt, h) in enumerate(runs):
                    msg = msgp.tile([128, max_run, 128], bf16, tag="msg",
                                    name=f"msg{l}_{r}")
                    vm_t = vmp.tile([128, max_run * 128], fp8, tag="vm",
                                    name=f"vm{l}_{r}")
                    nc.sync.dma_start(
                        out=vm_t[:, :nt * 128],
                        in_=vm_in[:, t0 * 128:(t0 + nt) * 128])
                    if "nogather" not in VAR:
                        # single_packet amortizes SDMA per-packet overhead but
                        # caps a call at 64 descs/engine = 8 tiles; sub-gathers
                        # share the run's queue so buffer/queue sems align
                        sp = "nosp" not in VAR
                        step = 8 if sp else nt
                        for s0 in range(0, nt, step):
                            sn = min(step, nt - s0)
                            nc.gpsimd.dma_gather(
                                out_ap=msg[:, s0:s0 + sn, :],
                                in_ap=(xwA[:] if h == 0 else xwB[:]),
                                idxs_ap=meta_idx[:, (t0 + s0) * 8:
                                                 (t0 + s0 + sn) * 8],
                                num_idxs=sn * 128,
                                num_idxs_reg=sn * 128,
                                elem_size=128,
                                single_packet=sp,
                                queue_num=(gg % nsq),
                            )
                            gg += 1
                    for j in range(nt):
                        t = t0 + j
                        w = int(tile_win[t])
                        if t == wfirst[h, w]:
                            cur = ps_seg.tile([128, 64], f32, tag="seg",
                                              name=f"seg{l}_{h}_{w}")
                            win_ps[w] = cur
                            if h == 0 or wfirst[0, w] < 0:
                                # open with bias: psum = sqrt(deg) x bias
                                nc.tensor.matmul(
                                    out=cur[:],
                                    lhsT=sqdeg_t[:, w * 128:(w + 1) * 128],
                                    rhs=b_tiles[l][:],
                                    start=True, stop=False,
                                )
                                # self-loop term: += dinv[i] * xw[i]
                                nc.tensor.matmul(
                                    out=cur[:],
                                    lhsT=ident_t[:],
                                    rhs=xw_stage[:, w * 64:(w + 1) * 64],
                                    start=False, stop=False,
                                )
                            else:
                                # re-inject parked phase-A sum
                                nc.tensor.matmul(
                                    out=cur[:],
                                    lhsT=ident_t[:],
                                    rhs=acc_t[:, w * 64:(w + 1) * 64],
                                    start=True, stop=False,
                                )
                        cur_ps = win_ps[w]
                        last = (t == wlast[h, w])
                        if "nomm" not in VAR:
                            nc.tensor.matmul(
                                out=cur_ps[:],
                                lhsT=vm_t[:, j * 128:(j + 1) * 128],
                                rhs=msg[:, j, 0:64],
                                start=False, stop=last,
                            )
                        elif last:
                            nc.scalar.copy(out=cur_ps[:], in_=cur_ps[:])
                        if last:
                            if h == 0 and wlast[1, w] >= 0:
                                # park phase-A sum in SBUF
                                nc.scalar.copy(
                                    out=acc_t[:, w * 64:(w + 1) * 64],
                                    in_=cur_ps[:])
                            else:
                                epilogue(w, cur_ps)

                # windows with no tiles at all (pad safety)
                for w in range(NW):
                    if wlast[0, w] < 0 and wlast[1, w] < 0:
                        cur = ps_seg.tile([128, 64], f32, tag="seg",
                                          name=f"segz{l}_{w}")
                        nc.tensor.matmul(
                            out=cur[:],
                            lhsT=sqdeg_t[:, w * 128:(w + 1) * 128],
                            rhs=b_tiles[l][:],
                            start=True, stop=False,
                        )
                        nc.tensor.matmul(
                            out=cur[:],
                            lhsT=ident_t[:],
                            rhs=xw_stage[:, w * 64:(w + 1) * 64],
                            start=False, stop=True,
                        )
                        epilogue(w, cur)

                hT_cur = hT_next

    nc.compile()
    return nc


def kernel(**inputs):
    from concourse import bass_utils

    x = np.asarray(inputs["x"], dtype=np.float32)
    edge_index = np.asarray(inputs["edge_index"])
    agent_idx = np.asarray(inputs["agent_idx"], dtype=np.int64)
    Ws = [np.asarray(inputs[f"W{i}"], dtype=np.float32) for i in range(3)]
    bs = [np.asarray(inputs[f"b{i}"], dtype=np.float32) for i in range(3)]

    idx_arr, vm8, dinv_own, sqdeg_own, sched = _preprocess(edge_index)

    nc = _build_program(sched)

    xpad = np.zeros((N_CORES * NPC, D), np.float32)
    xpad[:N_NODES] = x
    Wstack = np.ascontiguousarray(
        np.stack(Ws)).astype(ml_dtypes.bfloat16)
    bias_stack = np.ascontiguousarray(
        np.stack([b[None, :] for b in bs])).astype(ml_dtypes.bfloat16)
    ident = np.eye(128, dtype=ml_dtypes.bfloat16)

    in_maps = []
    for c in range(N_CORES):
        in_maps.append({
            "xT_own": np.ascontiguousarray(
                xpad[c * NPC:(c + 1) * NPC].T).astype(ml_dtypes.bfloat16),
            "src_idx": np.ascontiguousarray(idx_arr[c]),
            "vm8": np.ascontiguousarray(vm8[c]),
            "dinv_own": np.ascontiguousarray(dinv_own[c]),
            "sqdeg_own": np.ascontiguousarray(sqdeg_own[c]),
            "Wmat": Wstack,
            "bias_r": bias_stack,
            "ident": ident,
        })

    res = bass_utils.run_bass_kernel_spmd(
        nc, in_maps, core_ids=list(range(N_CORES)))

    taps = np.stack([np.asarray(res.results[c]["agents_out"])
                     .astype(np.float32) for c in range(N_CORES)])
    # taps[c, l, r, :] = h_l for node (c*NPC + 4*r)
    n_agents = agent_idx.shape[0]
    out = np.empty((n_agents, 3 * D), np.float32)
    c_of = agent_idx // NPC
    r_of = (agent_idx % NPC) // 4
    for l in range(3):
        out[:, l * D:(l + 1) * D] = taps[c_of, l, r_of, :]
    return out
